# revision 1
# baseline (speedup 1.0000x reference)
"""GRU cell kernel for Trainium2, data-parallel over 8 NeuronCores.

Reference computation (B=4096, I=H=2048, C=I+H=4096):
    combined   = [x, h]                                   [B, C]
    to_update  = sigmoid(combined @ W_update.T + b_u)     [B, H]
    to_select  = sigmoid(combined @ W_select.T + b_s)     [B, H]
    updated    = h * to_update
    new_comb   = [x, updated]
    predictions= tanh(new_comb @ W_predict.T + b_p)
    h_new      = h * (1 - to_select) + predictions * to_select

Sharding: batch split 8 ways (512 rows/core), weights replicated.
On-chip layout is [feature, batch] (transposed), so each weight tile
[128c, 128h] is the stationary matmul operand and activation tiles
[128c, 512b] are the moving operand -- no on-chip transposes anywhere.
Matmuls run in bf16 (inputs host-cast) with fp32 PSUM accumulation;
gates and the final blend stay fp32.
"""

from contextlib import ExitStack

import numpy as np
import ml_dtypes

import concourse.bass as bass
import concourse.tile as tile
import concourse.mybir as mybir
from concourse import bacc
from concourse.bass_utils import run_bass_kernel_spmd

BF16 = mybir.dt.bfloat16
F32 = mybir.dt.float32
NPBF16 = ml_dtypes.bfloat16

B, I, H = 4096, 2048, 2048
C = I + H
NCORES = 8
BS = B // NCORES            # 512 batch rows per core
P = 128                     # SBUF partitions
HT = H // P                 # 16 output-row tiles
IT = I // P                 # 16 x feature tiles
CT = C // P                 # 32 contraction tiles
ACT_F = mybir.ActivationFunctionType

_PROGRAM = None


def _build_program():
    nc = bacc.Bacc("TRN2")

    xT = nc.dram_tensor("xT", [IT, P, BS], BF16, kind="ExternalInput")
    hT32 = nc.dram_tensor("hT32", [HT, P, BS], F32, kind="ExternalInput")
    Wu = nc.dram_tensor("Wu", [HT, P, C], BF16, kind="ExternalInput")
    Ws = nc.dram_tensor("Ws", [HT, P, C], BF16, kind="ExternalInput")
    Wp = nc.dram_tensor("Wp", [HT, P, C], BF16, kind="ExternalInput")
    bu = nc.dram_tensor("bu", [P, HT], F32, kind="ExternalInput")
    bsel = nc.dram_tensor("bsel", [P, HT], F32, kind="ExternalInput")
    bp = nc.dram_tensor("bp", [P, HT], F32, kind="ExternalInput")
    out = nc.dram_tensor("out", [HT, P, BS], F32, kind="ExternalOutput")

    with tile.TileContext(nc) as tc, ExitStack() as ctx:
        singles = ctx.enter_context(tc.tile_pool(name="singles", bufs=1))
        wpool = ctx.enter_context(tc.tile_pool(name="wpool", bufs=4))
        pspool = ctx.enter_context(tc.tile_pool(name="ps", bufs=8, space="PSUM"))
        work = ctx.enter_context(tc.tile_pool(name="work", bufs=4))

        bu_sb = singles.tile([P, HT], F32, name="bu_sb")
        nc.sync.dma_start(bu_sb[:], bu[:])
        bs_sb = singles.tile([P, HT], F32, name="bs_sb")
        nc.sync.dma_start(bs_sb[:], bsel[:])
        bp_sb = singles.tile([P, HT], F32, name="bp_sb")
        nc.sync.dma_start(bp_sb[:], bp[:])

        # combined.T tiles: 16 x-tiles then 16 h-tiles (all bf16 [128, 512])
        comb = []
        for n in range(IT):
            t = singles.tile([P, BS], BF16, name=f"combx{n}", tag=f"combx{n}")
            nc.sync.dma_start(t[:], xT[n])
            comb.append(t)
        # h arrives fp32 (needed for the final blend); bf16 copies are cast
        # on-chip to avoid a second HBM upload of h.
        h32 = []
        for i in range(HT):
            t = singles.tile([P, BS], F32, name=f"h32_{i}", tag=f"h32_{i}")
            nc.sync.dma_start(t[:], hT32[i])
            h32.append(t)
        for i in range(HT):
            t = singles.tile([P, BS], BF16, name=f"combh{i}", tag=f"combh{i}")
            nc.vector.tensor_copy(t[:], h32[i][:])
            comb.append(t)

        upd = [
            singles.tile([P, BS], BF16, name=f"upd{i}", tag=f"upd{i}")
            for i in range(HT)
        ]
        sel = [
            singles.tile([P, BS], F32, name=f"sel{i}", tag=f"sel{i}")
            for i in range(HT)
        ]

        def gemm(W, rhs_tiles, i):
            """psum[128h, 512b] = sum_c W_tile[i].T @ rhs  (bf16, fp32 accum)"""
            wblk = wpool.tile([P, C], BF16, tag="wblk", name="wblk")
            half = C // 2
            nc.sync.dma_start(wblk[:, 0:half], W[i, :, 0:half])
            nc.sync.dma_start(wblk[:, half:C], W[i, :, half:C])
            ps = pspool.tile([P, BS], F32, tag="ps", name="ps")
            for n in range(CT):
                nc.tensor.matmul(
                    ps,
                    wblk[:, n * P:(n + 1) * P],
                    rhs_tiles[n],
                    start=(n == 0),
                    stop=(n == CT - 1),
                )
            return ps

        # update gate -> updated = h * sigmoid(z_u)  (bf16, feeds matmul 3)
        for i in range(HT):
            ps = gemm(Wu, comb, i)
            u = work.tile([P, BS], BF16, tag="u", name="u")
            nc.scalar.activation(u[:], ps[:], ACT_F.Sigmoid, bias=bu_sb[:, i:i + 1])
            nc.vector.tensor_mul(upd[i][:], comb[IT + i][:], u[:])

        # select gate (fp32, used in final blend)
        for i in range(HT):
            ps = gemm(Ws, comb, i)
            nc.scalar.activation(
                sel[i][:], ps[:], ACT_F.Sigmoid, bias=bs_sb[:, i:i + 1]
            )

        # predictions + blend: h_new = h + sel * (tanh(z_p) - h)
        newcomb = comb[:IT] + upd
        for i in range(HT):
            ps = gemm(Wp, newcomb, i)
            p_t = work.tile([P, BS], F32, tag="p", name="p_t")
            nc.scalar.activation(p_t[:], ps[:], ACT_F.Tanh, bias=bp_sb[:, i:i + 1])
            d = work.tile([P, BS], F32, tag="d", name="d")
            nc.vector.tensor_sub(d[:], p_t[:], h32[i][:])
            nc.vector.tensor_mul(d[:], d[:], sel[i][:])
            o = work.tile([P, BS], F32, tag="o", name="o")
            nc.vector.tensor_add(o[:], h32[i][:], d[:])
            nc.sync.dma_start(out[i], o[:])

    nc.finalize()
    return nc


def _get_program():
    global _PROGRAM
    if _PROGRAM is None:
        _PROGRAM = _build_program()
    return _PROGRAM


def _pack_weight(w):
    """[H, C] fp32 -> [HT, P, C] bf16 with [i, p, n*128+m] = W[i*128+m, n*128+p].

    Slice [i] is then an SBUF block whose column window n*128:(n+1)*128 is the
    stationary operand (lhsT = W.T tile) for contraction tile n.
    """
    wb = np.asarray(w, dtype=np.float32).astype(NPBF16)
    return np.ascontiguousarray(
        wb.reshape(HT, P, CT, P).transpose(0, 3, 2, 1).reshape(HT, P, C)
    )


def _prep_inputs(x, h, W_update, b_update, W_select, b_select, W_predict, b_predict):
    x = np.asarray(x, dtype=np.float32)
    h = np.asarray(h, dtype=np.float32)

    Wu = _pack_weight(W_update)
    Ws = _pack_weight(W_select)
    Wp = _pack_weight(W_predict)
    bu = np.ascontiguousarray(
        np.asarray(b_update, dtype=np.float32).reshape(HT, P).T
    )
    bsel = np.ascontiguousarray(
        np.asarray(b_select, dtype=np.float32).reshape(HT, P).T
    )
    bp = np.ascontiguousarray(
        np.asarray(b_predict, dtype=np.float32).reshape(HT, P).T
    )

    in_maps = []
    for c in range(NCORES):
        rows = slice(c * BS, (c + 1) * BS)
        xT = np.ascontiguousarray(x[rows].T.astype(NPBF16).reshape(IT, P, BS))
        hT32 = np.ascontiguousarray(h[rows].T.reshape(HT, P, BS))
        in_maps.append(
            {
                "xT": xT,
                "hT32": hT32,
                "Wu": Wu,
                "Ws": Ws,
                "Wp": Wp,
                "bu": bu,
                "bsel": bsel,
                "bp": bp,
            }
        )
    return in_maps


def kernel(x, h, W_update, b_update, W_select, b_select, W_predict, b_predict,
           _trace=False):
    nc = _get_program()
    in_maps = _prep_inputs(
        x, h, W_update, b_update, W_select, b_select, W_predict, b_predict
    )
    res = run_bass_kernel_spmd(
        nc, in_maps, core_ids=list(range(NCORES)), trace=_trace
    )
    h_new = np.empty((B, H), dtype=np.float32)
    for c in range(NCORES):
        rows = slice(c * BS, (c + 1) * BS)
        h_new[rows] = res.results[c]["out"].reshape(H, BS).T
    if _trace:
        return h_new, res
    return h_new



# revision 20
# speedup vs baseline: 1.4197x; 1.4197x over previous
"""GRU cell kernel for Trainium2, data-parallel over 8 NeuronCores.

Reference computation (B=4096, I=H=2048, C=I+H=4096):
    combined   = [x, h]                                   [B, C]
    to_update  = sigmoid(combined @ W_update.T + b_u)     [B, H]
    to_select  = sigmoid(combined @ W_select.T + b_s)     [B, H]
    updated    = h * to_update
    new_comb   = [x, updated]
    predictions= tanh(new_comb @ W_predict.T + b_p)
    h_new      = h * (1 - to_select) + predictions * to_select

Sharding: batch split 8 ways (512 rows/core), weights replicated.
On-chip layout is [feature, batch] (transposed), so each weight tile
is the stationary matmul operand and activation tiles [128c, 512b] are
the moving operand -- no on-chip transposes anywhere.

GEMMs run in fp8e4m3 DoubleRow perf mode (2 contraction sub-tiles per
instruction) with split precision: every operand T is stored as
T = T_hi + T_lo (two fp8 tensors, shared scale) and each product is
computed as hi*hi + hi*lo + lo*hi (lo*lo dropped), which recovers
~bf16 accuracy at 0.75x the bf16 matmul cost. Weights are pre-scaled
by 64 so their values (std 1/64) land in fp8's normal range; the 1/64
is folded into the activation instruction's input scale. PSUM
accumulation is fp32; gates and the final blend run in bf16/fp32.
"""

from contextlib import ExitStack

import numpy as np
import ml_dtypes

import concourse.bass as bass
import concourse.tile as tile
import concourse.mybir as mybir
from concourse import bacc
from concourse.bass_utils import run_bass_kernel_spmd

F8 = mybir.dt.float8e4
BF16 = mybir.dt.bfloat16
F32 = mybir.dt.float32
NPF8 = ml_dtypes.float8_e4m3
NPBF16 = ml_dtypes.bfloat16

B, I, H = 4096, 2048, 2048
C = I + H
NCORES = 8
BS = B // NCORES            # 512 batch rows per core
P = 128                     # SBUF partitions
HT = H // P                 # 16 output-row tiles
IT = I // P                 # 16 x feature tiles
CT = C // P                 # 32 contraction tiles
CP = CT // 2                # 16 DoubleRow contraction pairs
SW = 64.0                   # weight quantization scale (2^6)
ACT_F = mybir.ActivationFunctionType
DR = mybir.MatmulPerfMode.DoubleRow

PHASED_START = True         # tiles 0-3 of gemm1 as three phased sweeps
TAIL_CHUNKS = 4             # last gemm3 tile split into this many psum chunks

_PROGRAM = None


def _build_program():
    nc = bacc.Bacc("TRN2")

    xhi = nc.dram_tensor("xhi", [P, IT, BS], F8, kind="ExternalInput")
    xlo = nc.dram_tensor("xlo", [P, IT, BS], F8, kind="ExternalInput")
    hhi = nc.dram_tensor("hhi", [P, HT, BS], F8, kind="ExternalInput")
    hlo = nc.dram_tensor("hlo", [P, HT, BS], F8, kind="ExternalInput")
    hbf = nc.dram_tensor("hbf", [P, HT, BS], BF16, kind="ExternalInput")
    Wuh = nc.dram_tensor("Wuh", [HT, P, C], F8, kind="ExternalInput")
    Wul = nc.dram_tensor("Wul", [HT, P, C], F8, kind="ExternalInput")
    Wsh = nc.dram_tensor("Wsh", [HT, P, C], F8, kind="ExternalInput")
    Wsl = nc.dram_tensor("Wsl", [HT, P, C], F8, kind="ExternalInput")
    Wph = nc.dram_tensor("Wph", [HT, P, C], F8, kind="ExternalInput")
    Wpl = nc.dram_tensor("Wpl", [HT, P, C], F8, kind="ExternalInput")
    bu = nc.dram_tensor("bu", [P, HT], F32, kind="ExternalInput")
    bsel = nc.dram_tensor("bsel", [P, HT], F32, kind="ExternalInput")
    bp = nc.dram_tensor("bp", [P, HT], F32, kind="ExternalInput")
    out = nc.dram_tensor("out", [HT, P, BS], BF16, kind="ExternalOutput")

    with tile.TileContext(nc) as tc, ExitStack() as ctx:
        singles = ctx.enter_context(tc.tile_pool(name="singles", bufs=1))
        wpool = ctx.enter_context(tc.tile_pool(name="wpool", bufs=4))
        pspool = ctx.enter_context(tc.tile_pool(name="ps", bufs=8, space="PSUM"))
        work = ctx.enter_context(tc.tile_pool(name="work", bufs=4))

        # combined.T fp8 hi/lo, one [P, 32, 512] tile each; c-tiles 0..15
        # are x, 16..31 are h. Upload order is the startup critical path:
        # first gemm's weights, then hi acts, then lo acts, all in 256KB-ish
        # chunks so the first matmuls can start as soon as their chunk lands.
        comb_hi = singles.tile([P, CT, BS], F8, name="comb_hi")
        comb_lo = singles.tile([P, CT, BS], F8, name="comb_lo")
        # Startup DMA order = the cold-start critical path. Tiny first
        # chunks so the first matmul can start ~3us in; then ~256KB chunks
        # (the HWDGE issue rate bounds anything smaller). hi weights for
        # tiles 0-3 come before any lo data: tiles 0-3 run as three phased
        # sweeps (hihi x4 tiles, then lohi x4, then hilo x4, four psum banks
        # held open) so the PE has runnable work for most of the initial
        # comb upload instead of stalling on tile 0's full contraction.
        whi_t = [
            wpool.tile([P, CT, P], F8, tag="whi", name=f"whi{i}") for i in range(4)
        ]
        wlo_t = [
            wpool.tile([P, CT, P], F8, tag="wlo", name=f"wlo{i}") for i in range(4)
        ]
        nc.sync.dma_start(whi_t[0][:, 0:2, :], Wuh[0, :, 0:2 * P])
        nc.sync.dma_start(comb_hi[:, 0:2, :], xhi[:, 0:2, :])
        nc.sync.dma_start(whi_t[0][:, 2:16, :], Wuh[0, :, 2 * P:16 * P])
        nc.sync.dma_start(comb_hi[:, 2:6, :], xhi[:, 2:6, :])
        nc.sync.dma_start(whi_t[0][:, 16:CT, :], Wuh[0, :, 16 * P:C])
        nc.sync.dma_start(comb_hi[:, 6:11, :], xhi[:, 6:11, :])
        nc.sync.dma_start(comb_hi[:, 11:16, :], xhi[:, 11:16, :])
        for k in range(4):
            nc.sync.dma_start(
                comb_hi[:, IT + 4 * k:IT + 4 * k + 4, :], hhi[:, 4 * k:4 * k + 4, :]
            )
        for i in range(1, 4):
            nc.sync.dma_start(whi_t[i][:], Wuh[i])
        for i in range(4):
            nc.sync.dma_start(wlo_t[i][:], Wul[i])
        for k in range(4):
            nc.sync.dma_start(comb_lo[:, 4 * k:4 * k + 4, :], xlo[:, 4 * k:4 * k + 4, :])
        for k in range(4):
            nc.sync.dma_start(
                comb_lo[:, IT + 4 * k:IT + 4 * k + 4, :], hlo[:, 4 * k:4 * k + 4, :]
            )

        # biases + bf16 h ride behind the fp8 uploads (first needed by the
        # tile-0 update gate, ~20us in; later hb chunks are issued inside
        # the gemm1 loop, before their first reader, so they queue behind
        # the next tiles' weight streams).
        bu_sb = singles.tile([P, HT], F32, name="bu_sb")
        nc.sync.dma_start(bu_sb[:], bu[:])
        bs_sb = singles.tile([P, HT], F32, name="bs_sb")
        nc.sync.dma_start(bs_sb[:], bsel[:])
        bp_sb = singles.tile([P, HT], F32, name="bp_sb")
        nc.sync.dma_start(bp_sb[:], bp[:])
        hb = singles.tile([P, HT, BS], BF16, name="hb")
        nc.sync.dma_start(hb[:, 0:4, :], hbf[:, 0:4, :])

        updhi = singles.tile([P, HT, BS], F8, name="updhi")
        updlo = singles.tile([P, HT, BS], F8, name="updlo")
        selt = singles.tile([P, HT, BS], BF16, name="selt")
        keept = singles.tile([P, HT, BS], BF16, name="keept")

        def mov12(hi, n, cols):
            src = comb_hi if hi else comb_lo
            return src[:, 2 * n:2 * n + 2, cols]

        def mov3(hi, n, cols):
            if n < IT // 2:
                src = comb_hi if hi else comb_lo
                return src[:, 2 * n:2 * n + 2, cols]
            m = n - IT // 2
            src = updhi if hi else updlo
            return src[:, 2 * m:2 * m + 2, cols]

        def gemm(Wh, Wl, i, mov, pre=None, cols=slice(0, BS)):
            """psum[128h, 512b] = sum_c (W.T @ comb) via fp8 DoubleRow.
            Sweep order hi*hi, lo*hi, hi*lo: the lo activations are the last
            DMA to land at startup, so their sweep goes last."""
            if pre is not None:
                whi, wlo = pre
            else:
                whi = wpool.tile([P, CT, P], F8, tag="whi", name="whi")
                nc.sync.dma_start(whi[:], Wh[i])
                wlo = wpool.tile([P, CT, P], F8, tag="wlo", name="wlo")
                nc.sync.dma_start(wlo[:], Wl[i])
            ncols = cols.stop - cols.start
            if ncols == BS:
                ps = pspool.tile([P, BS], F32, tag="ps", name="ps", bufs=6)
            else:
                ps = pspool.tile([P, ncols], F32, tag="pshalf", name="pshalf", bufs=2)
            for n in range(CP):
                nc.tensor.matmul(
                    ps, whi[:, 2 * n:2 * n + 2, :], mov(True, n, cols),
                    start=(n == 0), stop=False, perf_mode=DR,
                )
            for n in range(CP):
                nc.tensor.matmul(
                    ps, wlo[:, 2 * n:2 * n + 2, :], mov(True, n, cols),
                    start=False, stop=False, perf_mode=DR,
                )
            for n in range(CP):
                nc.tensor.matmul(
                    ps, whi[:, 2 * n:2 * n + 2, :], mov(False, n, cols),
                    start=False, stop=(n == CP - 1), perf_mode=DR,
                )
            return ps, whi, wlo

        # update gate -> updated = h * sigmoid(z_u), split to fp8 hi/lo
        # (feeds gemm3's moving operand)
        def upd_split(ps, i):
            u = work.tile([P, BS], BF16, tag="u", name="u")
            nc.scalar.activation(
                u[:], ps[:], ACT_F.Sigmoid, bias=bu_sb[:, i:i + 1], scale=1.0 / SW
            )
            upd32 = work.tile([P, BS], F32, tag="upd32", name="upd32")
            nc.vector.tensor_mul(upd32[:], hb[:, i, :], u[:])
            nc.vector.tensor_copy(updhi[:, i, :], upd32[:])
            back = work.tile([P, BS], F32, tag="back", name="back")
            nc.scalar.activation(back[:], updhi[:, i, :], ACT_F.Copy)
            nc.vector.tensor_sub(updlo[:, i, :], upd32[:], back[:])

        if PHASED_START:
            # tiles 0-3: phased sweeps over four open psum banks
            ps_t = [
                pspool.tile([P, BS], F32, tag="ps", name="ps", bufs=6)
                for i in range(4)
            ]
            for i in range(4):
                for n in range(CP):
                    nc.tensor.matmul(
                        ps_t[i], whi_t[i][:, 2 * n:2 * n + 2, :],
                        mov12(True, n, slice(0, BS)),
                        start=(n == 0), stop=False, perf_mode=DR,
                    )
            for i in range(4):
                for n in range(CP):
                    nc.tensor.matmul(
                        ps_t[i], wlo_t[i][:, 2 * n:2 * n + 2, :],
                        mov12(True, n, slice(0, BS)),
                        start=False, stop=False, perf_mode=DR,
                    )
            for i in range(4):
                for n in range(CP):
                    nc.tensor.matmul(
                        ps_t[i], whi_t[i][:, 2 * n:2 * n + 2, :],
                        mov12(False, n, slice(0, BS)),
                        start=False, stop=(n == CP - 1), perf_mode=DR,
                    )
                upd_split(ps_t[i], i)
        else:
            for i in range(4):
                ps, _, _ = gemm(Wuh, Wul, i, mov12, pre=(whi_t[i], wlo_t[i]))
                upd_split(ps, i)

        for i in range(4, HT):
            ps, _, _ = gemm(Wuh, Wul, i, mov12)
            if i <= 6:
                k = i - 3
                nc.sync.dma_start(
                    hb[:, 4 * k:4 * k + 4, :], hbf[:, 4 * k:4 * k + 4, :]
                )
            upd_split(ps, i)

        # select gate; precompute keep = h*(1-sel) so the gemm3 tail is short
        for i in range(HT):
            ps, _, _ = gemm(Wsh, Wsl, i, mov12)
            nc.scalar.activation(
                selt[:, i, :], ps[:], ACT_F.Sigmoid,
                bias=bs_sb[:, i:i + 1], scale=1.0 / SW,
            )
            hs = work.tile([P, BS], BF16, tag="hs", name="hs")
            nc.vector.tensor_mul(hs[:], hb[:, i, :], selt[:, i, :])
            nc.vector.tensor_sub(keept[:, i, :], hb[:, i, :], hs[:])

        # predictions + blend: h_new = keep + tanh(z_p) * sel. The last
        # tile runs as two half-width psum groups so the final blend chain
        # overlaps the final matmuls (shorter drain after the last matmul).
        def blend_tail(ps, i, cols):
            p_t = work.tile([P, BS], BF16, tag="p", name="p_t")
            nc.scalar.activation(
                p_t[:, cols], ps[:], ACT_F.Tanh,
                bias=bp_sb[:, i:i + 1], scale=1.0 / SW,
            )
            ps2 = work.tile([P, BS], BF16, tag="ps2", name="ps2")
            nc.vector.tensor_mul(ps2[:, cols], p_t[:, cols], selt[:, i, cols])
            o = work.tile([P, BS], BF16, tag="o", name="o")
            nc.vector.tensor_add(o[:, cols], ps2[:, cols], keept[:, i, cols])
            nc.sync.dma_start(out[i, :, cols], o[:, cols])

        for i in range(HT - 1):
            ps, _, _ = gemm(Wph, Wpl, i, mov3)
            blend_tail(ps, i, slice(0, BS))
        i = HT - 1
        pre = None
        for q in range(TAIL_CHUNKS):
            cols = slice(q * BS // TAIL_CHUNKS, (q + 1) * BS // TAIL_CHUNKS)
            ps_q, whi_l, wlo_l = gemm(Wph, Wpl, i, mov3, pre=pre, cols=cols)
            pre = (whi_l, wlo_l)
            blend_tail(ps_q, i, cols)

    nc.finalize()
    return nc


def _get_program():
    global _PROGRAM
    if _PROGRAM is None:
        _PROGRAM = _build_program()
    return _PROGRAM


def _split8(a):
    """fp32 array -> (hi, lo) float8_e4m3 with hi + lo ~= a."""
    hi = a.astype(NPF8)
    lo = (a - hi.astype(np.float32)).astype(NPF8)
    return hi, lo


def _pack_weight(w):
    """[H, C] fp8 -> [HT, P, C] with [i, p, n*128+m] = w[i*128+m, n*128+p].

    Slice [i] is an SBUF block whose column window n*128:(n+1)*128 is the
    stationary operand (lhsT = W.T tile) for contraction tile n.
    """
    return np.ascontiguousarray(
        w.reshape(HT, P, CT, P).transpose(0, 3, 2, 1).reshape(HT, P, C)
    )


def _prep_inputs(x, h, W_update, b_update, W_select, b_select, W_predict, b_predict):
    x = np.asarray(x, dtype=np.float32)
    h = np.asarray(h, dtype=np.float32)

    packed_w = {}
    for name, w in (("Wu", W_update), ("Ws", W_select), ("Wp", W_predict)):
        ws = np.asarray(w, dtype=np.float32) * np.float32(SW)
        whi, wlo = _split8(ws)
        packed_w[name + "h"] = _pack_weight(whi)
        packed_w[name + "l"] = _pack_weight(wlo)

    bu = np.ascontiguousarray(
        np.asarray(b_update, dtype=np.float32).reshape(HT, P).T
    )
    bsel = np.ascontiguousarray(
        np.asarray(b_select, dtype=np.float32).reshape(HT, P).T
    )
    bp = np.ascontiguousarray(
        np.asarray(b_predict, dtype=np.float32).reshape(HT, P).T
    )

    xT = np.ascontiguousarray(x.T)          # [I, B]
    hT = np.ascontiguousarray(h.T)          # [H, B]
    xT_hi, xT_lo = _split8(xT)
    hT_hi, hT_lo = _split8(hT)
    hT_bf = hT.astype(NPBF16)

    def pmaj(a, cols, nt):
        """[F, B] host slice -> [P, nt, BS] partition-major dram layout."""
        return np.ascontiguousarray(
            a[:, cols].reshape(nt, P, BS).transpose(1, 0, 2)
        )

    in_maps = []
    for c in range(NCORES):
        cols = slice(c * BS, (c + 1) * BS)
        in_maps.append(
            {
                "xhi": pmaj(xT_hi, cols, IT),
                "xlo": pmaj(xT_lo, cols, IT),
                "hhi": pmaj(hT_hi, cols, HT),
                "hlo": pmaj(hT_lo, cols, HT),
                "hbf": pmaj(hT_bf, cols, HT),
                "Wuh": packed_w["Wuh"],
                "Wul": packed_w["Wul"],
                "Wsh": packed_w["Wsh"],
                "Wsl": packed_w["Wsl"],
                "Wph": packed_w["Wph"],
                "Wpl": packed_w["Wpl"],
                "bu": bu,
                "bsel": bsel,
                "bp": bp,
            }
        )
    return in_maps


def kernel(x, h, W_update, b_update, W_select, b_select, W_predict, b_predict,
           _trace=False):
    nc = _get_program()
    in_maps = _prep_inputs(
        x, h, W_update, b_update, W_select, b_select, W_predict, b_predict
    )
    res = run_bass_kernel_spmd(
        nc, in_maps, core_ids=list(range(NCORES)), trace=_trace
    )
    h_new = np.empty((B, H), dtype=np.float32)
    for c in range(NCORES):
        rows = slice(c * BS, (c + 1) * BS)
        h_new[rows] = res.results[c]["out"].reshape(H, BS).T
    if _trace:
        return h_new, res
    return h_new


# revision 27
# speedup vs baseline: 1.5657x; 1.1028x over previous
"""GRU cell kernel for Trainium2, data-parallel over 8 NeuronCores.

Reference computation (B=4096, I=H=2048, C=I+H=4096):
    combined   = [x, h]                                   [B, C]
    to_update  = sigmoid(combined @ W_update.T + b_u)     [B, H]
    to_select  = sigmoid(combined @ W_select.T + b_s)     [B, H]
    updated    = h * to_update
    new_comb   = [x, updated]
    predictions= tanh(new_comb @ W_predict.T + b_p)
    h_new      = h * (1 - to_select) + predictions * to_select

Sharding: batch split 8 ways (512 rows/core), weights replicated.
On-chip layout is [feature, batch] (transposed), so each weight tile
is the stationary matmul operand and activation tiles [128c, 512b] are
the moving operand -- no on-chip transposes anywhere.

GEMMs run in fp8e4m3 DoubleRow perf mode (2 contraction sub-tiles per
instruction) with split precision: every operand T is stored as
T = T_hi + T_lo (two fp8 tensors, shared scale) and each product is
computed as hi*hi + hi*lo + lo*hi (lo*lo dropped), which recovers
~bf16 accuracy at 0.75x the bf16 matmul cost. Weights are pre-scaled
by 64 so their values (std 1/64) land in fp8's normal range; the 1/64
is folded into the activation instruction's input scale. PSUM
accumulation is fp32; gates and the final blend run in bf16/fp32.
"""

from contextlib import ExitStack

import numpy as np
import ml_dtypes

import concourse.bass as bass
import concourse.tile as tile
import concourse.mybir as mybir
from concourse import bacc
from concourse.bass_utils import run_bass_kernel_spmd

F8 = mybir.dt.float8e4
BF16 = mybir.dt.bfloat16
F32 = mybir.dt.float32
NPF8 = ml_dtypes.float8_e4m3
NPBF16 = ml_dtypes.bfloat16

B, I, H = 4096, 2048, 2048
C = I + H
NCORES = 8
BS = B // NCORES            # 512 batch rows per core
P = 128                     # SBUF partitions
HT = H // P                 # 16 output-row tiles
IT = I // P                 # 16 x feature tiles
CT = C // P                 # 32 contraction tiles
CP = CT // 2                # 16 DoubleRow contraction pairs
SW = 64.0                   # weight quantization scale (2^6)
ACT_F = mybir.ActivationFunctionType
DR = mybir.MatmulPerfMode.DoubleRow

PHASED_START = True         # tiles 0-3 of gemm1 as three phased sweeps
TAIL_CHUNKS = 4             # last gemm3 tile split into this many psum chunks

_PROGRAM = None


def _build_program():
    nc = bacc.Bacc("TRN2")

    xhi = nc.dram_tensor("xhi", [P, IT, BS], F8, kind="ExternalInput")
    xlo = nc.dram_tensor("xlo", [P, IT, BS], F8, kind="ExternalInput")
    hhi = nc.dram_tensor("hhi", [P, HT, BS], F8, kind="ExternalInput")
    hlo = nc.dram_tensor("hlo", [P, HT, BS], F8, kind="ExternalInput")
    hbf = nc.dram_tensor("hbf", [P, HT, BS], BF16, kind="ExternalInput")
    Wuh = nc.dram_tensor("Wuh", [HT, P, C], F8, kind="ExternalInput")
    Wul = nc.dram_tensor("Wul", [HT, P, C], F8, kind="ExternalInput")
    Wsh = nc.dram_tensor("Wsh", [HT, P, C], F8, kind="ExternalInput")
    Wsl = nc.dram_tensor("Wsl", [HT, P, C], F8, kind="ExternalInput")
    Wph = nc.dram_tensor("Wph", [HT, P, C], F8, kind="ExternalInput")
    Wpl = nc.dram_tensor("Wpl", [HT, P, C], F8, kind="ExternalInput")
    bu = nc.dram_tensor("bu", [P, HT], F32, kind="ExternalInput")
    bsel = nc.dram_tensor("bsel", [P, HT], F32, kind="ExternalInput")
    bp = nc.dram_tensor("bp", [P, HT], F32, kind="ExternalInput")
    out = nc.dram_tensor("out", [HT, P, BS], BF16, kind="ExternalOutput")

    with tile.TileContext(nc) as tc, ExitStack() as ctx:
        singles = ctx.enter_context(tc.tile_pool(name="singles", bufs=1))
        wpool = ctx.enter_context(tc.tile_pool(name="wpool", bufs=4))
        pspool = ctx.enter_context(tc.tile_pool(name="ps", bufs=8, space="PSUM"))
        work = ctx.enter_context(tc.tile_pool(name="work", bufs=4))

        # combined.T fp8 hi/lo, one [P, 32, 512] tile each; c-tiles 0..15
        # are x, 16..31 are h. Upload order is the startup critical path:
        # first gemm's weights, then hi acts, then lo acts, all in 256KB-ish
        # chunks so the first matmuls can start as soon as their chunk lands.
        comb_hi = singles.tile([P, CT, BS], F8, name="comb_hi")
        comb_lo = singles.tile([P, CT, BS], F8, name="comb_lo")
        # Startup DMA order = the cold-start critical path. Tiny first
        # chunks so the first matmul can start ~3us in; then ~256KB chunks
        # (the HWDGE issue rate bounds anything smaller). hi weights for
        # tiles 0-3 come before any lo data: tiles 0-3 run as three phased
        # sweeps (hihi x4 tiles, then lohi x4, then hilo x4, four psum banks
        # held open) so the PE has runnable work for most of the initial
        # comb upload instead of stalling on tile 0's full contraction.
        whi_t = [
            wpool.tile([P, CT, P], F8, tag="whi", name=f"whi{i}") for i in range(4)
        ]
        wlo_t = [
            wpool.tile([P, CT, P], F8, tag="wlo", name=f"wlo{i}") for i in range(4)
        ]
        nc.sync.dma_start(whi_t[0][:, 0:2, :], Wuh[0, :, 0:2 * P])
        nc.sync.dma_start(comb_hi[:, 0:2, :], xhi[:, 0:2, :])
        nc.sync.dma_start(whi_t[0][:, 2:16, :], Wuh[0, :, 2 * P:16 * P])
        nc.sync.dma_start(comb_hi[:, 2:6, :], xhi[:, 2:6, :])
        nc.sync.dma_start(whi_t[0][:, 16:CT, :], Wuh[0, :, 16 * P:C])
        nc.sync.dma_start(comb_hi[:, 6:11, :], xhi[:, 6:11, :])
        nc.sync.dma_start(comb_hi[:, 11:16, :], xhi[:, 11:16, :])
        for k in range(4):
            nc.sync.dma_start(
                comb_hi[:, IT + 4 * k:IT + 4 * k + 4, :], hhi[:, 4 * k:4 * k + 4, :]
            )
        for i in range(1, 4):
            nc.sync.dma_start(whi_t[i][:], Wuh[i])
        for i in range(4):
            nc.sync.dma_start(wlo_t[i][:], Wul[i])
        # comb_lo is first needed by gemm2 (~60us in): its chunks are issued
        # inside the gemm1 loop so the gemm1 weight stream isn't delayed.

        # biases + bf16 h ride behind the fp8 uploads (first needed by the
        # tile-0 update gate, ~20us in; later hb chunks are issued inside
        # the gemm1 loop, before their first reader, so they queue behind
        # the next tiles' weight streams).
        bu_sb = singles.tile([P, HT], F32, name="bu_sb")
        nc.sync.dma_start(bu_sb[:], bu[:])
        bs_sb = singles.tile([P, HT], F32, name="bs_sb")
        nc.sync.dma_start(bs_sb[:], bsel[:])
        bp_sb = singles.tile([P, HT], F32, name="bp_sb")
        nc.sync.dma_start(bp_sb[:], bp[:])
        hb = singles.tile([P, HT, BS], BF16, name="hb")
        nc.sync.dma_start(hb[:, 0:4, :], hbf[:, 0:4, :])

        updhi = singles.tile([P, HT, BS], F8, name="updhi")
        updlo = singles.tile([P, HT, BS], F8, name="updlo")
        selt = singles.tile([P, HT, BS], BF16, name="selt")
        keept = singles.tile([P, HT, BS], BF16, name="keept")

        def mov12(hi, n, cols):
            src = comb_hi if hi else comb_lo
            return src[:, 2 * n:2 * n + 2, cols]

        def mov3(hi, n, cols):
            if n < IT // 2:
                src = comb_hi if hi else comb_lo
                return src[:, 2 * n:2 * n + 2, cols]
            m = n - IT // 2
            src = updhi if hi else updlo
            return src[:, 2 * m:2 * m + 2, cols]

        def gemm(Wh, Wl, i, mov, pre=None, cols=slice(0, BS), products=3):
            """psum[128h, 512b] = sum_c (W.T @ comb) via fp8 DoubleRow.
            products=3: hi*hi + lo*hi + hi*lo (the lo-activations sweep goes
            last: they are the last DMA to land at startup). products=2
            drops the activations-lo sweep (update gate tolerates it)."""
            if pre is not None:
                whi, wlo = pre
            else:
                whi = wpool.tile([P, CT, P], F8, tag="whi", name="whi")
                nc.sync.dma_start(whi[:], Wh[i])
                wlo = wpool.tile([P, CT, P], F8, tag="wlo", name="wlo")
                nc.sync.dma_start(wlo[:], Wl[i])
            ncols = cols.stop - cols.start
            if ncols == BS:
                ps = pspool.tile([P, BS], F32, tag="ps", name="ps", bufs=6)
            else:
                ps = pspool.tile([P, ncols], F32, tag="pshalf", name="pshalf", bufs=2)
            for n in range(CP):
                nc.tensor.matmul(
                    ps, whi[:, 2 * n:2 * n + 2, :], mov(True, n, cols),
                    start=(n == 0), stop=False, perf_mode=DR,
                )
            for n in range(CP):
                nc.tensor.matmul(
                    ps, wlo[:, 2 * n:2 * n + 2, :], mov(True, n, cols),
                    start=False, stop=(products == 2 and n == CP - 1), perf_mode=DR,
                )
            if products == 3:
                for n in range(CP):
                    nc.tensor.matmul(
                        ps, whi[:, 2 * n:2 * n + 2, :], mov(False, n, cols),
                        start=False, stop=(n == CP - 1), perf_mode=DR,
                    )
            return ps, whi, wlo

        # update gate -> updated = h * sigmoid(z_u), split to fp8 hi/lo
        # (feeds gemm3's moving operand)
        def upd_split(ps, i):
            u = work.tile([P, BS], BF16, tag="u", name="u")
            nc.scalar.activation(
                u[:], ps[:], ACT_F.Sigmoid, bias=bu_sb[:, i:i + 1], scale=1.0 / SW
            )
            upd32 = work.tile([P, BS], F32, tag="upd32", name="upd32")
            nc.vector.tensor_mul(upd32[:], hb[:, i, :], u[:])
            nc.vector.tensor_copy(updhi[:, i, :], upd32[:])
            back = work.tile([P, BS], F32, tag="back", name="back")
            nc.scalar.activation(back[:], updhi[:, i, :], ACT_F.Copy)
            nc.vector.tensor_sub(updlo[:, i, :], upd32[:], back[:])

        if PHASED_START:
            # tiles 0-3: two phased sweeps over four open psum banks
            ps_t = [
                pspool.tile([P, BS], F32, tag="ps", name="ps", bufs=6)
                for i in range(4)
            ]
            for i in range(4):
                for n in range(CP):
                    nc.tensor.matmul(
                        ps_t[i], whi_t[i][:, 2 * n:2 * n + 2, :],
                        mov12(True, n, slice(0, BS)),
                        start=(n == 0), stop=False, perf_mode=DR,
                    )
            for i in range(4):
                for n in range(CP):
                    nc.tensor.matmul(
                        ps_t[i], wlo_t[i][:, 2 * n:2 * n + 2, :],
                        mov12(True, n, slice(0, BS)),
                        start=False, stop=(n == CP - 1), perf_mode=DR,
                    )
                upd_split(ps_t[i], i)
        else:
            for i in range(4):
                ps, _, _ = gemm(
                    Wuh, Wul, i, mov12, pre=(whi_t[i], wlo_t[i]), products=2
                )
                upd_split(ps, i)

        # hb chunk k is read by upd_split(4k); comb_lo is first read by
        # gemm2 tile 0 (needs all of it -- full contraction). Both stream
        # through the gemm1 loop's spare DMA bandwidth, spread thin so the
        # 1MB/tile gemm1 weight stream isn't starved.
        _hb_at = {4: 1, 7: 2, 10: 3}
        _lo_at = {6: 0, 8: 1, 9: 2, 11: 3, 12: 4, 13: 5, 14: 6, 15: 7}
        for i in range(4, HT):
            ps, _, _ = gemm(Wuh, Wul, i, mov12, products=2)
            if i in _hb_at:
                k = _hb_at[i]
                nc.sync.dma_start(
                    hb[:, 4 * k:4 * k + 4, :], hbf[:, 4 * k:4 * k + 4, :]
                )
            if i in _lo_at:
                k = _lo_at[i]
                if k < 4:
                    nc.sync.dma_start(
                        comb_lo[:, 4 * k:4 * k + 4, :], xlo[:, 4 * k:4 * k + 4, :]
                    )
                else:
                    k -= 4
                    nc.sync.dma_start(
                        comb_lo[:, IT + 4 * k:IT + 4 * k + 4, :],
                        hlo[:, 4 * k:4 * k + 4, :],
                    )
            upd_split(ps, i)

        # select gate; precompute keep = h*(1-sel) so the gemm3 tail is short
        for i in range(HT):
            ps, _, _ = gemm(Wsh, Wsl, i, mov12)
            nc.scalar.activation(
                selt[:, i, :], ps[:], ACT_F.Sigmoid,
                bias=bs_sb[:, i:i + 1], scale=1.0 / SW,
            )
            hs = work.tile([P, BS], BF16, tag="hs", name="hs")
            nc.vector.tensor_mul(hs[:], hb[:, i, :], selt[:, i, :])
            nc.vector.tensor_sub(keept[:, i, :], hb[:, i, :], hs[:])

        # predictions + blend: h_new = keep + tanh(z_p) * sel. The last
        # tile runs as two half-width psum groups so the final blend chain
        # overlaps the final matmuls (shorter drain after the last matmul).
        def blend_tail(ps, i, cols):
            p_t = work.tile([P, BS], BF16, tag="p", name="p_t")
            nc.scalar.activation(
                p_t[:, cols], ps[:], ACT_F.Tanh,
                bias=bp_sb[:, i:i + 1], scale=1.0 / SW,
            )
            ps2 = work.tile([P, BS], BF16, tag="ps2", name="ps2")
            nc.vector.tensor_mul(ps2[:, cols], p_t[:, cols], selt[:, i, cols])
            o = work.tile([P, BS], BF16, tag="o", name="o")
            nc.vector.tensor_add(o[:, cols], ps2[:, cols], keept[:, i, cols])
            nc.sync.dma_start(out[i, :, cols], o[:, cols])

        for i in range(HT - 1):
            ps, _, _ = gemm(Wph, Wpl, i, mov3)
            blend_tail(ps, i, slice(0, BS))
        i = HT - 1
        pre = None
        for q in range(TAIL_CHUNKS):
            cols = slice(q * BS // TAIL_CHUNKS, (q + 1) * BS // TAIL_CHUNKS)
            ps_q, whi_l, wlo_l = gemm(Wph, Wpl, i, mov3, pre=pre, cols=cols)
            pre = (whi_l, wlo_l)
            blend_tail(ps_q, i, cols)

    nc.finalize()
    return nc


def _get_program():
    global _PROGRAM
    if _PROGRAM is None:
        _PROGRAM = _build_program()
    return _PROGRAM


def _split8(a):
    """fp32 array -> (hi, lo) float8_e4m3 with hi + lo ~= a."""
    hi = a.astype(NPF8)
    lo = (a - hi.astype(np.float32)).astype(NPF8)
    return hi, lo


def _pack_weight(w):
    """[H, C] fp8 -> [HT, P, C] with [i, p, n*128+m] = w[i*128+m, n*128+p].

    Slice [i] is an SBUF block whose column window n*128:(n+1)*128 is the
    stationary operand (lhsT = W.T tile) for contraction tile n.
    """
    return np.ascontiguousarray(
        w.reshape(HT, P, CT, P).transpose(0, 3, 2, 1).reshape(HT, P, C)
    )


def _prep_inputs(x, h, W_update, b_update, W_select, b_select, W_predict, b_predict):
    x = np.asarray(x, dtype=np.float32)
    h = np.asarray(h, dtype=np.float32)

    packed_w = {}
    for name, w in (("Wu", W_update), ("Ws", W_select), ("Wp", W_predict)):
        ws = np.asarray(w, dtype=np.float32) * np.float32(SW)
        whi, wlo = _split8(ws)
        packed_w[name + "h"] = _pack_weight(whi)
        packed_w[name + "l"] = _pack_weight(wlo)

    bu = np.ascontiguousarray(
        np.asarray(b_update, dtype=np.float32).reshape(HT, P).T
    )
    bsel = np.ascontiguousarray(
        np.asarray(b_select, dtype=np.float32).reshape(HT, P).T
    )
    bp = np.ascontiguousarray(
        np.asarray(b_predict, dtype=np.float32).reshape(HT, P).T
    )

    xT = np.ascontiguousarray(x.T)          # [I, B]
    hT = np.ascontiguousarray(h.T)          # [H, B]
    xT_hi, xT_lo = _split8(xT)
    hT_hi, hT_lo = _split8(hT)
    hT_bf = hT.astype(NPBF16)

    def pmaj(a, cols, nt):
        """[F, B] host slice -> [P, nt, BS] partition-major dram layout."""
        return np.ascontiguousarray(
            a[:, cols].reshape(nt, P, BS).transpose(1, 0, 2)
        )

    in_maps = []
    for c in range(NCORES):
        cols = slice(c * BS, (c + 1) * BS)
        in_maps.append(
            {
                "xhi": pmaj(xT_hi, cols, IT),
                "xlo": pmaj(xT_lo, cols, IT),
                "hhi": pmaj(hT_hi, cols, HT),
                "hlo": pmaj(hT_lo, cols, HT),
                "hbf": pmaj(hT_bf, cols, HT),
                "Wuh": packed_w["Wuh"],
                "Wul": packed_w["Wul"],
                "Wsh": packed_w["Wsh"],
                "Wsl": packed_w["Wsl"],
                "Wph": packed_w["Wph"],
                "Wpl": packed_w["Wpl"],
                "bu": bu,
                "bsel": bsel,
                "bp": bp,
            }
        )
    return in_maps


def kernel(x, h, W_update, b_update, W_select, b_select, W_predict, b_predict,
           _trace=False):
    nc = _get_program()
    in_maps = _prep_inputs(
        x, h, W_update, b_update, W_select, b_select, W_predict, b_predict
    )
    res = run_bass_kernel_spmd(
        nc, in_maps, core_ids=list(range(NCORES)), trace=_trace
    )
    h_new = np.empty((B, H), dtype=np.float32)
    for c in range(NCORES):
        rows = slice(c * BS, (c + 1) * BS)
        h_new[rows] = res.results[c]["out"].reshape(H, BS).T
    if _trace:
        return h_new, res
    return h_new


# revision 29
# speedup vs baseline: 1.5660x; 1.0002x over previous
"""GRU cell kernel for Trainium2, data-parallel over 8 NeuronCores.

Reference computation (B=4096, I=H=2048, C=I+H=4096):
    combined   = [x, h]                                   [B, C]
    to_update  = sigmoid(combined @ W_update.T + b_u)     [B, H]
    to_select  = sigmoid(combined @ W_select.T + b_s)     [B, H]
    updated    = h * to_update
    new_comb   = [x, updated]
    predictions= tanh(new_comb @ W_predict.T + b_p)
    h_new      = h * (1 - to_select) + predictions * to_select

Sharding: batch split 8 ways (512 rows/core), weights replicated.
On-chip layout is [feature, batch] (transposed), so each weight tile
is the stationary matmul operand and activation tiles [128c, 512b] are
the moving operand -- no on-chip transposes anywhere.

GEMMs run in fp8e4m3 DoubleRow perf mode (2 contraction sub-tiles per
instruction) with split precision: every operand T is stored as
T = T_hi + T_lo (two fp8 tensors, shared scale) and each product is
computed as hi*hi + hi*lo + lo*hi (lo*lo dropped), which recovers
~bf16 accuracy at 0.75x the bf16 matmul cost. Weights are pre-scaled
by 64 so their values (std 1/64) land in fp8's normal range; the 1/64
is folded into the activation instruction's input scale. PSUM
accumulation is fp32; gates and the final blend run in bf16/fp32.
"""

from contextlib import ExitStack

import numpy as np
import ml_dtypes

import concourse.bass as bass
import concourse.tile as tile
import concourse.mybir as mybir
from concourse import bacc
from concourse.bass_utils import run_bass_kernel_spmd

F8 = mybir.dt.float8e4
BF16 = mybir.dt.bfloat16
F32 = mybir.dt.float32
NPF8 = ml_dtypes.float8_e4m3
NPBF16 = ml_dtypes.bfloat16

B, I, H = 4096, 2048, 2048
C = I + H
NCORES = 8
BS = B // NCORES            # 512 batch rows per core
P = 128                     # SBUF partitions
HT = H // P                 # 16 output-row tiles
IT = I // P                 # 16 x feature tiles
CT = C // P                 # 32 contraction tiles
CP = CT // 2                # 16 DoubleRow contraction pairs
SW = 64.0                   # weight quantization scale (2^6)
ACT_F = mybir.ActivationFunctionType
DR = mybir.MatmulPerfMode.DoubleRow

PHASED_START = True         # tiles 0-3 of gemm1 as three phased sweeps
TAIL_CHUNKS = 4             # last gemm3 tile split into this many psum chunks

_PROGRAM = None


def _build_program():
    nc = bacc.Bacc("TRN2")

    xhi = nc.dram_tensor("xhi", [P, IT, BS], F8, kind="ExternalInput")
    xlo = nc.dram_tensor("xlo", [P, IT, BS], F8, kind="ExternalInput")
    hhi = nc.dram_tensor("hhi", [P, HT, BS], F8, kind="ExternalInput")
    hlo = nc.dram_tensor("hlo", [P, HT, BS], F8, kind="ExternalInput")
    hbf = nc.dram_tensor("hbf", [P, HT, BS], BF16, kind="ExternalInput")
    Wuh = nc.dram_tensor("Wuh", [HT, P, C], F8, kind="ExternalInput")
    Wul = nc.dram_tensor("Wul", [HT, P, C], F8, kind="ExternalInput")
    Wsh = nc.dram_tensor("Wsh", [HT, P, C], F8, kind="ExternalInput")
    Wsl = nc.dram_tensor("Wsl", [HT, P, C], F8, kind="ExternalInput")
    Wph = nc.dram_tensor("Wph", [HT, P, C], F8, kind="ExternalInput")
    Wpl = nc.dram_tensor("Wpl", [HT, P, C], F8, kind="ExternalInput")
    bu = nc.dram_tensor("bu", [P, HT], F32, kind="ExternalInput")
    bsel = nc.dram_tensor("bsel", [P, HT], F32, kind="ExternalInput")
    bp = nc.dram_tensor("bp", [P, HT], F32, kind="ExternalInput")
    out = nc.dram_tensor("out", [HT, P, BS], BF16, kind="ExternalOutput")

    with tile.TileContext(nc) as tc, ExitStack() as ctx:
        singles = ctx.enter_context(tc.tile_pool(name="singles", bufs=1))
        wpool = ctx.enter_context(tc.tile_pool(name="wpool", bufs=4))
        pspool = ctx.enter_context(tc.tile_pool(name="ps", bufs=8, space="PSUM"))
        work = ctx.enter_context(tc.tile_pool(name="work", bufs=4))

        # combined.T fp8 hi/lo, one [P, 32, 512] tile each; c-tiles 0..15
        # are x, 16..31 are h. Upload order is the startup critical path:
        # first gemm's weights, then hi acts, then lo acts, all in 256KB-ish
        # chunks so the first matmuls can start as soon as their chunk lands.
        comb_hi = singles.tile([P, CT, BS], F8, name="comb_hi")
        comb_lo = singles.tile([P, CT, BS], F8, name="comb_lo")
        # Startup DMA order = the cold-start critical path. Tiny first
        # chunks so the first matmul can start ~3us in; then ~256KB chunks
        # (the HWDGE issue rate bounds anything smaller). hi weights for
        # tiles 0-3 come before any lo data: tiles 0-3 run as three phased
        # sweeps (hihi x4 tiles, then lohi x4, then hilo x4, four psum banks
        # held open) so the PE has runnable work for most of the initial
        # comb upload instead of stalling on tile 0's full contraction.
        whi_t = [
            wpool.tile([P, CT, P], F8, tag="whi", name=f"whi{i}") for i in range(4)
        ]
        wlo_t = [
            wpool.tile([P, CT, P], F8, tag="wlo", name=f"wlo{i}") for i in range(4)
        ]
        # first two tiny chunks issue from different engine queues so their
        # HWDGE slots pipeline instead of waiting on one SEQ
        nc.sync.dma_start(whi_t[0][:, 0:2, :], Wuh[0, :, 0:2 * P])
        nc.scalar.dma_start(comb_hi[:, 0:2, :], xhi[:, 0:2, :])
        nc.sync.dma_start(whi_t[0][:, 2:16, :], Wuh[0, :, 2 * P:16 * P])
        nc.sync.dma_start(comb_hi[:, 2:6, :], xhi[:, 2:6, :])
        nc.sync.dma_start(whi_t[0][:, 16:CT, :], Wuh[0, :, 16 * P:C])
        nc.sync.dma_start(comb_hi[:, 6:11, :], xhi[:, 6:11, :])
        nc.sync.dma_start(comb_hi[:, 11:16, :], xhi[:, 11:16, :])
        for k in range(4):
            nc.sync.dma_start(
                comb_hi[:, IT + 4 * k:IT + 4 * k + 4, :], hhi[:, 4 * k:4 * k + 4, :]
            )
        for i in range(1, 4):
            nc.sync.dma_start(whi_t[i][:], Wuh[i])
        for i in range(4):
            nc.sync.dma_start(wlo_t[i][:], Wul[i])
        # comb_lo is first needed by gemm2 (~60us in): its chunks are issued
        # inside the gemm1 loop so the gemm1 weight stream isn't delayed.

        # biases + bf16 h ride behind the fp8 uploads (first needed by the
        # tile-0 update gate, ~20us in; later hb chunks are issued inside
        # the gemm1 loop, before their first reader, so they queue behind
        # the next tiles' weight streams).
        bu_sb = singles.tile([P, HT], F32, name="bu_sb")
        nc.sync.dma_start(bu_sb[:], bu[:])
        bs_sb = singles.tile([P, HT], F32, name="bs_sb")
        nc.sync.dma_start(bs_sb[:], bsel[:])
        bp_sb = singles.tile([P, HT], F32, name="bp_sb")
        nc.sync.dma_start(bp_sb[:], bp[:])
        hb = singles.tile([P, HT, BS], BF16, name="hb")
        nc.sync.dma_start(hb[:, 0:4, :], hbf[:, 0:4, :])

        updhi = singles.tile([P, HT, BS], F8, name="updhi")
        updlo = singles.tile([P, HT, BS], F8, name="updlo")
        selt = singles.tile([P, HT, BS], BF16, name="selt")
        keept = singles.tile([P, HT, BS], BF16, name="keept")

        def mov12(hi, n, cols):
            src = comb_hi if hi else comb_lo
            return src[:, 2 * n:2 * n + 2, cols]

        def mov3(hi, n, cols):
            if n < IT // 2:
                src = comb_hi if hi else comb_lo
                return src[:, 2 * n:2 * n + 2, cols]
            m = n - IT // 2
            src = updhi if hi else updlo
            return src[:, 2 * m:2 * m + 2, cols]

        def gemm(Wh, Wl, i, mov, pre=None, cols=slice(0, BS), products=3):
            """psum[128h, 512b] = sum_c (W.T @ comb) via fp8 DoubleRow.
            products=3: hi*hi + lo*hi + hi*lo (the lo-activations sweep goes
            last: they are the last DMA to land at startup). products=2
            drops the activations-lo sweep (update gate tolerates it)."""
            if pre is not None:
                whi, wlo = pre
            else:
                whi = wpool.tile([P, CT, P], F8, tag="whi", name="whi")
                nc.sync.dma_start(whi[:], Wh[i])
                wlo = wpool.tile([P, CT, P], F8, tag="wlo", name="wlo")
                nc.sync.dma_start(wlo[:], Wl[i])
            ncols = cols.stop - cols.start
            if ncols == BS:
                ps = pspool.tile([P, BS], F32, tag="ps", name="ps", bufs=6)
            else:
                ps = pspool.tile([P, ncols], F32, tag="pshalf", name="pshalf", bufs=2)
            for n in range(CP):
                nc.tensor.matmul(
                    ps, whi[:, 2 * n:2 * n + 2, :], mov(True, n, cols),
                    start=(n == 0), stop=False, perf_mode=DR,
                )
            for n in range(CP):
                nc.tensor.matmul(
                    ps, wlo[:, 2 * n:2 * n + 2, :], mov(True, n, cols),
                    start=False, stop=(products == 2 and n == CP - 1), perf_mode=DR,
                )
            if products == 3:
                for n in range(CP):
                    nc.tensor.matmul(
                        ps, whi[:, 2 * n:2 * n + 2, :], mov(False, n, cols),
                        start=False, stop=(n == CP - 1), perf_mode=DR,
                    )
            return ps, whi, wlo

        # update gate -> updated = h * sigmoid(z_u), split to fp8 hi/lo
        # (feeds gemm3's moving operand)
        def upd_split(ps, i):
            u = work.tile([P, BS], BF16, tag="u", name="u")
            nc.scalar.activation(
                u[:], ps[:], ACT_F.Sigmoid, bias=bu_sb[:, i:i + 1], scale=1.0 / SW
            )
            upd32 = work.tile([P, BS], F32, tag="upd32", name="upd32")
            nc.vector.tensor_mul(upd32[:], hb[:, i, :], u[:])
            nc.vector.tensor_copy(updhi[:, i, :], upd32[:])
            back = work.tile([P, BS], F32, tag="back", name="back")
            nc.scalar.activation(back[:], updhi[:, i, :], ACT_F.Copy)
            nc.vector.tensor_sub(updlo[:, i, :], upd32[:], back[:])

        if PHASED_START:
            # tiles 0-3: two phased sweeps over four open psum banks
            ps_t = [
                pspool.tile([P, BS], F32, tag="ps", name="ps", bufs=6)
                for i in range(4)
            ]
            for i in range(4):
                for n in range(CP):
                    nc.tensor.matmul(
                        ps_t[i], whi_t[i][:, 2 * n:2 * n + 2, :],
                        mov12(True, n, slice(0, BS)),
                        start=(n == 0), stop=False, perf_mode=DR,
                    )
            for i in range(4):
                for n in range(CP):
                    nc.tensor.matmul(
                        ps_t[i], wlo_t[i][:, 2 * n:2 * n + 2, :],
                        mov12(True, n, slice(0, BS)),
                        start=False, stop=(n == CP - 1), perf_mode=DR,
                    )
                upd_split(ps_t[i], i)
        else:
            for i in range(4):
                ps, _, _ = gemm(
                    Wuh, Wul, i, mov12, pre=(whi_t[i], wlo_t[i]), products=2
                )
                upd_split(ps, i)

        # hb chunk k is read by upd_split(4k); comb_lo is first read by
        # gemm2 tile 0 (needs all of it -- full contraction). Both stream
        # through the gemm1 loop's spare DMA bandwidth, spread thin so the
        # 1MB/tile gemm1 weight stream isn't starved.
        _hb_at = {4: 1, 7: 2, 10: 3}
        _lo_at = {6: 0, 8: 1, 9: 2, 11: 3, 12: 4, 13: 5, 14: 6, 15: 7}
        for i in range(4, HT):
            ps, _, _ = gemm(Wuh, Wul, i, mov12, products=2)
            if i in _hb_at:
                k = _hb_at[i]
                nc.sync.dma_start(
                    hb[:, 4 * k:4 * k + 4, :], hbf[:, 4 * k:4 * k + 4, :]
                )
            if i in _lo_at:
                k = _lo_at[i]
                if k < 4:
                    nc.sync.dma_start(
                        comb_lo[:, 4 * k:4 * k + 4, :], xlo[:, 4 * k:4 * k + 4, :]
                    )
                else:
                    k -= 4
                    nc.sync.dma_start(
                        comb_lo[:, IT + 4 * k:IT + 4 * k + 4, :],
                        hlo[:, 4 * k:4 * k + 4, :],
                    )
            upd_split(ps, i)

        # select gate; precompute keep = h*(1-sel) so the gemm3 tail is short
        for i in range(HT):
            ps, _, _ = gemm(Wsh, Wsl, i, mov12)
            nc.scalar.activation(
                selt[:, i, :], ps[:], ACT_F.Sigmoid,
                bias=bs_sb[:, i:i + 1], scale=1.0 / SW,
            )
            hs = work.tile([P, BS], BF16, tag="hs", name="hs")
            nc.vector.tensor_mul(hs[:], hb[:, i, :], selt[:, i, :])
            nc.vector.tensor_sub(keept[:, i, :], hb[:, i, :], hs[:])

        # predictions + blend: h_new = keep + tanh(z_p) * sel. The last
        # tile runs as two half-width psum groups so the final blend chain
        # overlaps the final matmuls (shorter drain after the last matmul).
        def blend_tail(ps, i, cols):
            p_t = work.tile([P, BS], BF16, tag="p", name="p_t")
            nc.scalar.activation(
                p_t[:, cols], ps[:], ACT_F.Tanh,
                bias=bp_sb[:, i:i + 1], scale=1.0 / SW,
            )
            ps2 = work.tile([P, BS], BF16, tag="ps2", name="ps2")
            nc.vector.tensor_mul(ps2[:, cols], p_t[:, cols], selt[:, i, cols])
            o = work.tile([P, BS], BF16, tag="o", name="o")
            nc.vector.tensor_add(o[:, cols], ps2[:, cols], keept[:, i, cols])
            nc.sync.dma_start(out[i, :, cols], o[:, cols])

        for i in range(HT - 1):
            ps, _, _ = gemm(Wph, Wpl, i, mov3)
            blend_tail(ps, i, slice(0, BS))
        i = HT - 1
        pre = None
        for q in range(TAIL_CHUNKS):
            cols = slice(q * BS // TAIL_CHUNKS, (q + 1) * BS // TAIL_CHUNKS)
            ps_q, whi_l, wlo_l = gemm(Wph, Wpl, i, mov3, pre=pre, cols=cols)
            pre = (whi_l, wlo_l)
            blend_tail(ps_q, i, cols)

    nc.finalize()
    return nc


def _get_program():
    global _PROGRAM
    if _PROGRAM is None:
        _PROGRAM = _build_program()
    return _PROGRAM


def _split8(a):
    """fp32 array -> (hi, lo) float8_e4m3 with hi + lo ~= a."""
    hi = a.astype(NPF8)
    lo = (a - hi.astype(np.float32)).astype(NPF8)
    return hi, lo


def _pack_weight(w):
    """[H, C] fp8 -> [HT, P, C] with [i, p, n*128+m] = w[i*128+m, n*128+p].

    Slice [i] is an SBUF block whose column window n*128:(n+1)*128 is the
    stationary operand (lhsT = W.T tile) for contraction tile n.
    """
    return np.ascontiguousarray(
        w.reshape(HT, P, CT, P).transpose(0, 3, 2, 1).reshape(HT, P, C)
    )


def _prep_inputs(x, h, W_update, b_update, W_select, b_select, W_predict, b_predict):
    x = np.asarray(x, dtype=np.float32)
    h = np.asarray(h, dtype=np.float32)

    packed_w = {}
    for name, w in (("Wu", W_update), ("Ws", W_select), ("Wp", W_predict)):
        ws = np.asarray(w, dtype=np.float32) * np.float32(SW)
        whi, wlo = _split8(ws)
        packed_w[name + "h"] = _pack_weight(whi)
        packed_w[name + "l"] = _pack_weight(wlo)

    bu = np.ascontiguousarray(
        np.asarray(b_update, dtype=np.float32).reshape(HT, P).T
    )
    bsel = np.ascontiguousarray(
        np.asarray(b_select, dtype=np.float32).reshape(HT, P).T
    )
    bp = np.ascontiguousarray(
        np.asarray(b_predict, dtype=np.float32).reshape(HT, P).T
    )

    xT = np.ascontiguousarray(x.T)          # [I, B]
    hT = np.ascontiguousarray(h.T)          # [H, B]
    xT_hi, xT_lo = _split8(xT)
    hT_hi, hT_lo = _split8(hT)
    hT_bf = hT.astype(NPBF16)

    def pmaj(a, cols, nt):
        """[F, B] host slice -> [P, nt, BS] partition-major dram layout."""
        return np.ascontiguousarray(
            a[:, cols].reshape(nt, P, BS).transpose(1, 0, 2)
        )

    in_maps = []
    for c in range(NCORES):
        cols = slice(c * BS, (c + 1) * BS)
        in_maps.append(
            {
                "xhi": pmaj(xT_hi, cols, IT),
                "xlo": pmaj(xT_lo, cols, IT),
                "hhi": pmaj(hT_hi, cols, HT),
                "hlo": pmaj(hT_lo, cols, HT),
                "hbf": pmaj(hT_bf, cols, HT),
                "Wuh": packed_w["Wuh"],
                "Wul": packed_w["Wul"],
                "Wsh": packed_w["Wsh"],
                "Wsl": packed_w["Wsl"],
                "Wph": packed_w["Wph"],
                "Wpl": packed_w["Wpl"],
                "bu": bu,
                "bsel": bsel,
                "bp": bp,
            }
        )
    return in_maps


def kernel(x, h, W_update, b_update, W_select, b_select, W_predict, b_predict,
           _trace=False):
    nc = _get_program()
    in_maps = _prep_inputs(
        x, h, W_update, b_update, W_select, b_select, W_predict, b_predict
    )
    res = run_bass_kernel_spmd(
        nc, in_maps, core_ids=list(range(NCORES)), trace=_trace
    )
    h_new = np.empty((B, H), dtype=np.float32)
    for c in range(NCORES):
        rows = slice(c * BS, (c + 1) * BS)
        h_new[rows] = res.results[c]["out"].reshape(H, BS).T
    if _trace:
        return h_new, res
    return h_new


# revision 39
# speedup vs baseline: 1.5933x; 1.0174x over previous
"""GRU cell kernel for Trainium2, data-parallel over 8 NeuronCores.

Reference computation (B=4096, I=H=2048, C=I+H=4096):
    combined   = [x, h]                                   [B, C]
    to_update  = sigmoid(combined @ W_update.T + b_u)     [B, H]
    to_select  = sigmoid(combined @ W_select.T + b_s)     [B, H]
    updated    = h * to_update
    new_comb   = [x, updated]
    predictions= tanh(new_comb @ W_predict.T + b_p)
    h_new      = h * (1 - to_select) + predictions * to_select

Sharding: batch split 8 ways (512 rows/core), weights replicated.
On-chip layout is [feature, batch] (transposed), so each weight tile
is the stationary matmul operand and activation tiles [128c, 512b] are
the moving operand -- no on-chip transposes anywhere.

GEMMs run in fp8e4m3 DoubleRow perf mode (2 contraction sub-tiles per
instruction) with split precision: every operand T is stored as
T = T_hi + T_lo (two fp8 tensors, shared scale) and each product is
computed as hi*hi + hi*lo + lo*hi (lo*lo dropped), which recovers
~bf16 accuracy at 0.75x the bf16 matmul cost. Weights are pre-scaled
by 64 so their values (std 1/64) land in fp8's normal range; the 1/64
is folded into the activation instruction's input scale. PSUM
accumulation is fp32; gates and the final blend run in bf16/fp32.
"""

from contextlib import ExitStack

import numpy as np
import ml_dtypes

import concourse.bass as bass
import concourse.tile as tile
import concourse.mybir as mybir
from concourse import bacc
from concourse.bass_utils import run_bass_kernel_spmd

F8 = mybir.dt.float8e4
BF16 = mybir.dt.bfloat16
F32 = mybir.dt.float32
NPF8 = ml_dtypes.float8_e4m3
NPBF16 = ml_dtypes.bfloat16

B, I, H = 4096, 2048, 2048
C = I + H
NCORES = 8
BS = B // NCORES            # 512 batch rows per core
P = 128                     # SBUF partitions
HT = H // P                 # 16 output-row tiles
IT = I // P                 # 16 x feature tiles
CT = C // P                 # 32 contraction tiles
CP = CT // 2                # 16 DoubleRow contraction pairs
SW = 64.0                   # weight quantization scale (2^6)
ACT_F = mybir.ActivationFunctionType
DR = mybir.MatmulPerfMode.DoubleRow

PHASED_START = True         # tiles 0-3 of gemm1 as three phased sweeps
TAIL_CHUNKS = 4             # last gemm3 tile split into this many psum chunks

_PROGRAM = None


def _build_program():
    nc = bacc.Bacc("TRN2")

    xhi = nc.dram_tensor("xhi", [P, IT, BS], F8, kind="ExternalInput")
    xlo = nc.dram_tensor("xlo", [P, IT, BS], F8, kind="ExternalInput")
    hhi = nc.dram_tensor("hhi", [P, HT, BS], F8, kind="ExternalInput")
    hlo = nc.dram_tensor("hlo", [P, HT, BS], F8, kind="ExternalInput")
    Wuh = nc.dram_tensor("Wuh", [HT, P, C], F8, kind="ExternalInput")
    Wul = nc.dram_tensor("Wul", [HT, P, C], F8, kind="ExternalInput")
    Wsh = nc.dram_tensor("Wsh", [HT, P, C], F8, kind="ExternalInput")
    Wsl = nc.dram_tensor("Wsl", [HT, P, C], F8, kind="ExternalInput")
    Wph = nc.dram_tensor("Wph", [HT, P, C], F8, kind="ExternalInput")
    Wpl = nc.dram_tensor("Wpl", [HT, P, C], F8, kind="ExternalInput")
    bu = nc.dram_tensor("bu", [P, HT], F32, kind="ExternalInput")
    bsel = nc.dram_tensor("bsel", [P, HT], F32, kind="ExternalInput")
    bp = nc.dram_tensor("bp", [P, HT], F32, kind="ExternalInput")
    out = nc.dram_tensor("out", [HT, P, BS], BF16, kind="ExternalOutput")

    with tile.TileContext(nc) as tc, ExitStack() as ctx:
        singles = ctx.enter_context(tc.tile_pool(name="singles", bufs=1))
        wpool = ctx.enter_context(tc.tile_pool(name="wpool", bufs=4))
        pspool = ctx.enter_context(tc.tile_pool(name="ps", bufs=8, space="PSUM"))
        work = ctx.enter_context(tc.tile_pool(name="work", bufs=4))

        # combined.T fp8 hi/lo, one [P, 32, 512] tile each; c-tiles 0..15
        # are x, 16..31 are h. Upload order is the startup critical path:
        # first gemm's weights, then hi acts, then lo acts, all in 256KB-ish
        # chunks so the first matmuls can start as soon as their chunk lands.
        comb_hi = singles.tile([P, CT, BS], F8, name="comb_hi")
        comb_lo = singles.tile([P, CT, BS], F8, name="comb_lo")
        # Startup DMA order = the cold-start critical path. Tiny first
        # chunks so the first matmul can start ~3us in; then ~256KB chunks
        # (the HWDGE issue rate bounds anything smaller). hi weights for
        # tiles 0-3 come before any lo data: tiles 0-3 run as three phased
        # sweeps (hihi x4 tiles, then lohi x4, then hilo x4, four psum banks
        # held open) so the PE has runnable work for most of the initial
        # comb upload instead of stalling on tile 0's full contraction.
        whi_t = [
            wpool.tile([P, CT, P], F8, tag="whi", name=f"whi{i}") for i in range(4)
        ]
        wlo_t = [
            wpool.tile([P, CT, P], F8, tag="wlo", name=f"wlo{i}") for i in range(4)
        ]
        # first two tiny chunks issue from different engine queues so their
        # HWDGE slots pipeline instead of waiting on one SEQ
        nc.sync.dma_start(whi_t[0][:, 0:2, :], Wuh[0, :, 0:2 * P])
        nc.scalar.dma_start(comb_hi[:, 0:2, :], xhi[:, 0:2, :])
        nc.sync.dma_start(whi_t[0][:, 2:16, :], Wuh[0, :, 2 * P:16 * P])
        nc.sync.dma_start(comb_hi[:, 2:6, :], xhi[:, 2:6, :])
        nc.sync.dma_start(whi_t[0][:, 16:CT, :], Wuh[0, :, 16 * P:C])
        nc.sync.dma_start(comb_hi[:, 6:11, :], xhi[:, 6:11, :])
        nc.sync.dma_start(comb_hi[:, 11:16, :], xhi[:, 11:16, :])
        for k in range(4):
            nc.sync.dma_start(
                comb_hi[:, IT + 4 * k:IT + 4 * k + 4, :], hhi[:, 4 * k:4 * k + 4, :]
            )
        for i in range(1, 4):
            nc.sync.dma_start(whi_t[i][:], Wuh[i])
        for i in range(4):
            nc.sync.dma_start(wlo_t[i][:], Wul[i])
        # comb_lo is first needed by gemm2 (~60us in): its chunks are issued
        # inside the gemm1 loop so the gemm1 weight stream isn't delayed.

        # biases + bf16 h ride behind the fp8 uploads (first needed by the
        # tile-0 update gate, ~20us in; later hb chunks are issued inside
        # the gemm1 loop, before their first reader, so they queue behind
        # the next tiles' weight streams).
        bu_sb = singles.tile([P, HT], F32, name="bu_sb")
        nc.sync.dma_start(bu_sb[:], bu[:])
        bs_sb = singles.tile([P, HT], F32, name="bs_sb")
        nc.sync.dma_start(bs_sb[:], bsel[:])
        bp_sb = singles.tile([P, HT], F32, name="bp_sb")
        nc.sync.dma_start(bp_sb[:], bp[:])
        # h is reconstructed on-chip per tile as h_hi + h_lo (the fp8
        # split is already resident for the matmuls; bf16 sum == the old
        # uploaded-bf16-h precision) -- saves a 2MB h upload entirely.
        hrec = singles.tile([P, HT, BS], BF16, name="hrec")

        updhi = singles.tile([P, HT, BS], F8, name="updhi")
        updlo = singles.tile([P, HT, BS], F8, name="updlo")
        selt = singles.tile([P, HT, BS], BF16, name="selt")
        keept = singles.tile([P, HT, BS], BF16, name="keept")

        def mov12(hi, n, cols):
            src = comb_hi if hi else comb_lo
            return src[:, 2 * n:2 * n + 2, cols]

        def mov3(hi, n, cols):
            if n < IT // 2:
                src = comb_hi if hi else comb_lo
                return src[:, 2 * n:2 * n + 2, cols]
            m = n - IT // 2
            src = updhi if hi else updlo
            return src[:, 2 * m:2 * m + 2, cols]

        def gemm(Wh, Wl, i, mov, pre=None, cols=slice(0, BS), products=3):
            """psum[128h, 512b] = sum_c (W.T @ comb) via fp8 DoubleRow.
            products=3: hi*hi + lo*hi + hi*lo (the lo-activations sweep goes
            last: they are the last DMA to land at startup). products=2
            drops the activations-lo sweep (update gate tolerates it)."""
            if pre is not None:
                whi, wlo = pre
            else:
                whi = wpool.tile([P, CT, P], F8, tag="whi", name="whi")
                nc.sync.dma_start(whi[:], Wh[i])
                wlo = wpool.tile([P, CT, P], F8, tag="wlo", name="wlo")
                nc.sync.dma_start(wlo[:], Wl[i])
            ncols = cols.stop - cols.start
            if ncols == BS:
                ps = pspool.tile([P, BS], F32, tag="ps", name="ps", bufs=6)
            else:
                ps = pspool.tile([P, ncols], F32, tag="pshalf", name="pshalf", bufs=2)
            for n in range(CP):
                nc.tensor.matmul(
                    ps, whi[:, 2 * n:2 * n + 2, :], mov(True, n, cols),
                    start=(n == 0), stop=False, perf_mode=DR,
                )
            for n in range(CP):
                nc.tensor.matmul(
                    ps, wlo[:, 2 * n:2 * n + 2, :], mov(True, n, cols),
                    start=False, stop=(products == 2 and n == CP - 1), perf_mode=DR,
                )
            if products == 3:
                for n in range(CP):
                    nc.tensor.matmul(
                        ps, whi[:, 2 * n:2 * n + 2, :], mov(False, n, cols),
                        start=False, stop=(n == CP - 1), perf_mode=DR,
                    )
            return ps, whi, wlo

        # update gate -> updated = h * sigmoid(z_u), split to fp8 hi/lo
        # (feeds gemm3's moving operand)
        def upd_split(ps, i):
            nc.vector.tensor_add(
                hrec[:, i, :], comb_hi[:, IT + i, :], comb_lo[:, IT + i, :]
            )
            u = work.tile([P, BS], BF16, tag="u", name="u")
            nc.scalar.activation(
                u[:], ps[:], ACT_F.Sigmoid, bias=bu_sb[:, i:i + 1], scale=1.0 / SW
            )
            upd32 = work.tile([P, BS], F32, tag="upd32", name="upd32")
            nc.vector.tensor_mul(upd32[:], hrec[:, i, :], u[:])
            nc.vector.tensor_copy(updhi[:, i, :], upd32[:])
            back = work.tile([P, BS], F32, tag="back", name="back")
            nc.scalar.activation(back[:], updhi[:, i, :], ACT_F.Copy)
            nc.vector.tensor_sub(updlo[:, i, :], upd32[:], back[:])

        if PHASED_START:
            # tiles 0-3: two phased sweeps over four open psum banks
            ps_t = [
                pspool.tile([P, BS], F32, tag="ps", name="ps", bufs=6)
                for i in range(4)
            ]
            for i in range(4):
                for n in range(CP):
                    nc.tensor.matmul(
                        ps_t[i], whi_t[i][:, 2 * n:2 * n + 2, :],
                        mov12(True, n, slice(0, BS)),
                        start=(n == 0), stop=False, perf_mode=DR,
                    )
            for i in range(4):
                for n in range(CP):
                    nc.tensor.matmul(
                        ps_t[i], wlo_t[i][:, 2 * n:2 * n + 2, :],
                        mov12(True, n, slice(0, BS)),
                        start=False, stop=(n == CP - 1), perf_mode=DR,
                    )
        else:
            for i in range(4):
                ps, _, _ = gemm(
                    Wuh, Wul, i, mov12, pre=(whi_t[i], wlo_t[i]), products=2
                )
                upd_split(ps, i)

        # comb_lo h-chunk k is read by the hrec add of upd_split(4k); it
        # must ISSUE before that read (program order is what the tile
        # framework sequences on). Tiles 0-3's upd_splits are deferred into
        # iterations 4-7 (their psum banks stay live until then), so all
        # four h-lo chunks can ride the gemm1 loop's spare bandwidth; the
        # x-lo chunks are only read from gemm2 on and issue at its top.
        _lo_at = {4: (0, 1), 8: (2,), 12: (3,)}
        for i in range(4, HT):
            ps, _, _ = gemm(Wuh, Wul, i, mov12, products=2)
            for k in _lo_at.get(i, ()):
                nc.sync.dma_start(
                    comb_lo[:, IT + 4 * k:IT + 4 * k + 4, :],
                    hlo[:, 4 * k:4 * k + 4, :],
                )
            if i < 8:
                upd_split(ps_t[i - 4], i - 4)
            upd_split(ps, i)

        # select gate; precompute keep = h*(1-sel) so the gemm3 tail is short
        for i in range(HT):
            if i == 0:
                for k in range(4):
                    nc.sync.dma_start(
                        comb_lo[:, 4 * k:4 * k + 4, :], xlo[:, 4 * k:4 * k + 4, :]
                    )
            ps, _, _ = gemm(Wsh, Wsl, i, mov12)
            nc.scalar.activation(
                selt[:, i, :], ps[:], ACT_F.Sigmoid,
                bias=bs_sb[:, i:i + 1], scale=1.0 / SW,
            )
            hs = work.tile([P, BS], BF16, tag="hs", name="hs")
            nc.vector.tensor_mul(hs[:], hrec[:, i, :], selt[:, i, :])
            nc.vector.tensor_sub(keept[:, i, :], hrec[:, i, :], hs[:])

        # predictions + blend: h_new = keep + tanh(z_p) * sel. The last
        # tile runs as two half-width psum groups so the final blend chain
        # overlaps the final matmuls (shorter drain after the last matmul).
        def blend_tail(ps, i, cols):
            p_t = work.tile([P, BS], BF16, tag="p", name="p_t")
            nc.scalar.activation(
                p_t[:, cols], ps[:], ACT_F.Tanh,
                bias=bp_sb[:, i:i + 1], scale=1.0 / SW,
            )
            ps2 = work.tile([P, BS], BF16, tag="ps2", name="ps2")
            nc.vector.tensor_mul(ps2[:, cols], p_t[:, cols], selt[:, i, cols])
            o = work.tile([P, BS], BF16, tag="o", name="o")
            nc.vector.tensor_add(o[:, cols], ps2[:, cols], keept[:, i, cols])
            nc.sync.dma_start(out[i, :, cols], o[:, cols])

        for i in range(HT - 1):
            ps, _, _ = gemm(Wph, Wpl, i, mov3)
            blend_tail(ps, i, slice(0, BS))
        i = HT - 1
        pre = None
        for q in range(TAIL_CHUNKS):
            cols = slice(q * BS // TAIL_CHUNKS, (q + 1) * BS // TAIL_CHUNKS)
            ps_q, whi_l, wlo_l = gemm(Wph, Wpl, i, mov3, pre=pre, cols=cols)
            pre = (whi_l, wlo_l)
            blend_tail(ps_q, i, cols)

    nc.finalize()
    return nc


def _get_program():
    global _PROGRAM
    if _PROGRAM is None:
        _PROGRAM = _build_program()
    return _PROGRAM


def _split8(a):
    """fp32 array -> (hi, lo) float8_e4m3 with hi + lo ~= a."""
    hi = a.astype(NPF8)
    lo = (a - hi.astype(np.float32)).astype(NPF8)
    return hi, lo


def _pack_weight(w):
    """[H, C] fp8 -> [HT, P, C] with [i, p, n*128+m] = w[i*128+m, n*128+p].

    Slice [i] is an SBUF block whose column window n*128:(n+1)*128 is the
    stationary operand (lhsT = W.T tile) for contraction tile n.
    """
    return np.ascontiguousarray(
        w.reshape(HT, P, CT, P).transpose(0, 3, 2, 1).reshape(HT, P, C)
    )


def _prep_inputs(x, h, W_update, b_update, W_select, b_select, W_predict, b_predict):
    x = np.asarray(x, dtype=np.float32)
    h = np.asarray(h, dtype=np.float32)

    packed_w = {}
    for name, w in (("Wu", W_update), ("Ws", W_select), ("Wp", W_predict)):
        ws = np.asarray(w, dtype=np.float32) * np.float32(SW)
        whi, wlo = _split8(ws)
        packed_w[name + "h"] = _pack_weight(whi)
        packed_w[name + "l"] = _pack_weight(wlo)

    bu = np.ascontiguousarray(
        np.asarray(b_update, dtype=np.float32).reshape(HT, P).T
    )
    bsel = np.ascontiguousarray(
        np.asarray(b_select, dtype=np.float32).reshape(HT, P).T
    )
    bp = np.ascontiguousarray(
        np.asarray(b_predict, dtype=np.float32).reshape(HT, P).T
    )

    xT = np.ascontiguousarray(x.T)          # [I, B]
    hT = np.ascontiguousarray(h.T)          # [H, B]
    xT_hi, xT_lo = _split8(xT)
    hT_hi, hT_lo = _split8(hT)

    def pmaj(a, cols, nt):
        """[F, B] host slice -> [P, nt, BS] partition-major dram layout."""
        return np.ascontiguousarray(
            a[:, cols].reshape(nt, P, BS).transpose(1, 0, 2)
        )

    in_maps = []
    for c in range(NCORES):
        cols = slice(c * BS, (c + 1) * BS)
        in_maps.append(
            {
                "xhi": pmaj(xT_hi, cols, IT),
                "xlo": pmaj(xT_lo, cols, IT),
                "hhi": pmaj(hT_hi, cols, HT),
                "hlo": pmaj(hT_lo, cols, HT),
                "Wuh": packed_w["Wuh"],
                "Wul": packed_w["Wul"],
                "Wsh": packed_w["Wsh"],
                "Wsl": packed_w["Wsl"],
                "Wph": packed_w["Wph"],
                "Wpl": packed_w["Wpl"],
                "bu": bu,
                "bsel": bsel,
                "bp": bp,
            }
        )
    return in_maps


def kernel(x, h, W_update, b_update, W_select, b_select, W_predict, b_predict,
           _trace=False):
    nc = _get_program()
    in_maps = _prep_inputs(
        x, h, W_update, b_update, W_select, b_select, W_predict, b_predict
    )
    res = run_bass_kernel_spmd(
        nc, in_maps, core_ids=list(range(NCORES)), trace=_trace
    )
    h_new = np.empty((B, H), dtype=np.float32)
    for c in range(NCORES):
        rows = slice(c * BS, (c + 1) * BS)
        h_new[rows] = res.results[c]["out"].reshape(H, BS).T
    if _trace:
        return h_new, res
    return h_new
